# revision 1
# baseline (speedup 1.0000x reference)
"""Trainium2 Bass kernel for DeepSeek-style MoE gate routing.

hidden_states [8, 4096, 2048] f32, w [256, 2048] f32, bias [256] f32
 -> topk_idx [32768, 8] int32, topk_weight [32768, 8] f32

Sharding: tokens split 8 ways across NeuronCores (4096 tokens/core); the
small gate weight + bias are replicated.  x is pre-transposed on the host so
the hidden dim lands on SBUF partitions with fully-contiguous DMA.

Self-contained: hardcodes all shapes; only imports the concourse toolchain.
"""
import sys

if "/opt/trn_rl_repo" not in sys.path:
    sys.path.insert(0, "/opt/trn_rl_repo")

import numpy as np

import concourse.bass as bass  # noqa: F401  (bass must import before tile)
import concourse.mybir as mybir
import concourse.tile as tile
from concourse import bacc
from concourse.bass_utils import run_bass_kernel_spmd

P = 128            # partitions / tokens per tile
H = 2048           # hidden dim
E = 256            # experts
KO = H // P        # 16 contraction chunks
N_CORES = 8
T_CORE = 4096      # tokens per core
N_TILES = T_CORE // P      # 32 token tiles per core
ST_TOK = 512               # tokens per super-tile (DMA granularity)
N_ST = T_CORE // ST_TOK    # 8 super-tiles
TILES_PER_ST = ST_TOK // P  # 4

N_GROUP = 8
GSIZE = E // N_GROUP       # 32
TOPK_GROUP = 4
TOP_K = 8
SCALING = 2.5
NEG_BIG = -1.0e30

# "fp32": native fp32 matmul (4 cyc/row). "split3_fp16": 3x fp16 matmuls
# (hi*hi + scaled cross terms), ~fp32 precision at ~1/4 more PE cost than bf16.
MATMUL_MODE = "fp32"
LO_SCALE = 2048.0  # 2**11, keeps fp16 lo-parts in the normal range

f32 = mybir.dt.float32
f16 = mybir.dt.float16
u32 = mybir.dt.uint32
ALU = mybir.AluOpType
ACTF = mybir.ActivationFunctionType
AX = mybir.AxisListType

_CACHED_NC = {}


def build_kernel(mode=MATMUL_MODE):
    nc = bacc.Bacc("TRN2", target_bir_lowering=False, debug=False)

    if mode == "fp32":
        d_x = [nc.dram_tensor("xT", [H, T_CORE], f32, kind="ExternalInput")]
        d_w = [nc.dram_tensor("wT", [H, E], f32, kind="ExternalInput")]
    elif mode == "split3_fp16":
        d_x = [nc.dram_tensor("xTh", [H, T_CORE], f16, kind="ExternalInput"),
               nc.dram_tensor("xTl", [H, T_CORE], f16, kind="ExternalInput")]
        d_w = [nc.dram_tensor("wTh", [H, E], f16, kind="ExternalInput"),
               nc.dram_tensor("wTl", [H, E], f16, kind="ExternalInput")]
    else:
        raise ValueError(mode)
    d_bias = nc.dram_tensor("biasrep", [P, E], f32, kind="ExternalInput")
    d_oidx = nc.dram_tensor("oidx", [P, N_TILES, TOP_K], u32, kind="ExternalOutput")
    d_owgt = nc.dram_tensor("owgt", [P, N_TILES, TOP_K], f32, kind="ExternalOutput")

    xdt = f32 if mode == "fp32" else f16

    with tile.TileContext(nc) as tc:
        with tc.tile_pool(name="const", bufs=1) as cpool, \
             tc.tile_pool(name="xin", bufs=2) as xpool, \
             tc.tile_pool(name="score", bufs=3) as spool, \
             tc.tile_pool(name="small", bufs=3) as mpool, \
             tc.tile_pool(name="psum", bufs=4, space="PSUM") as ppool:

            # ---- constants ----
            w_sb = [cpool.tile([P, KO, E], w.dtype, name=f"w{i}")
                    for i, w in enumerate(d_w)]
            for t, d in zip(w_sb, d_w):
                nc.sync.dma_start(t, d.ap().rearrange("(ko p) e -> p ko e", p=P))
            bias_sb = cpool.tile([P, E], f32)
            nc.sync.dma_start(bias_sb, d_bias.ap())
            negbig = cpool.tile([P, 1], f32)
            nc.vector.memset(negbig, NEG_BIG)
            oidx_sb = cpool.tile([P, N_TILES, TOP_K], u32)
            owgt_sb = cpool.tile([P, N_TILES, TOP_K], f32)

            for st in range(N_ST):
                # ---- load xT super-tile: [128p, KO, 512t] per input tensor ----
                x_sb = []
                for i, d in enumerate(d_x):
                    t = xpool.tile([P, KO, ST_TOK], xdt, tag=f"x{i}")
                    nc.sync.dma_start(
                        t, d.ap().rearrange("(ko p) t -> p ko t", p=P)[
                            :, :, st * ST_TOK:(st + 1) * ST_TOK])
                    x_sb.append(t)

                for j in range(TILES_PER_ST):
                    tl = st * TILES_PER_ST + j
                    tsl = slice(j * P, (j + 1) * P)

                    # ---- logits: psum[128t, 256e] ----
                    if mode == "fp32":
                        ps = ppool.tile([P, E], f32, tag="ps")
                        for k in range(KO):
                            nc.tensor.matmul(
                                ps, lhsT=x_sb[0][:, k, tsl], rhs=w_sb[0][:, k, :],
                                start=(k == 0), stop=(k == KO - 1))
                        logits = ps  # PSUM fp32
                        sig_src = logits
                    else:
                        ps_hi = ppool.tile([P, E], f32, tag="ps")
                        ps_lo = ppool.tile([P, E], f32, tag="pslo")
                        xh, xl = x_sb
                        wh, wl = w_sb
                        for k in range(KO):
                            nc.tensor.matmul(
                                ps_hi, lhsT=xh[:, k, tsl], rhs=wh[:, k, :],
                                start=(k == 0), stop=(k == KO - 1))
                        for k in range(KO):
                            nc.tensor.matmul(
                                ps_lo, lhsT=xh[:, k, tsl], rhs=wl[:, k, :],
                                start=(k == 0), stop=False)
                            nc.tensor.matmul(
                                ps_lo, lhsT=xl[:, k, tsl], rhs=wh[:, k, :],
                                start=False, stop=(k == KO - 1))
                        # logits = ps_hi + ps_lo / LO_SCALE  (into SBUF)
                        logits = spool.tile([P, E], f32, tag="lg")
                        nc.vector.scalar_tensor_tensor(
                            logits, ps_lo, 1.0 / LO_SCALE, ps_hi,
                            op0=ALU.mult, op1=ALU.add)
                        sig_src = logits

                    # ---- sigma = sigmoid(logits) on ACT ----
                    sg = spool.tile([P, E], f32, tag="sg")
                    nc.scalar.activation(sg, sig_src, ACTF.Sigmoid)

                    # ---- scores_for_choice = sigma + bias ----
                    sb_ = spool.tile([P, E], f32, tag="sb")
                    nc.vector.tensor_add(sb_, sg, bias_sb)

                    # ---- per-group top-8 (only top-2 used) ----
                    gm = mpool.tile([P, N_GROUP, 8], f32, tag="gm")
                    for g in range(N_GROUP):
                        nc.vector.max(out=gm[:, g, :],
                                      in_=sb_[:, g * GSIZE:(g + 1) * GSIZE])
                    gs = mpool.tile([P, N_GROUP], f32, tag="gs")
                    nc.vector.tensor_add(gs, gm[:, :, 0], gm[:, :, 1])

                    # ---- group rank count: C[g] = #{m: gs_m > gs_g} ----
                    cc = mpool.tile([P, N_GROUP, N_GROUP], f32, tag="cc")
                    nc.vector.tensor_tensor(
                        out=cc,
                        in0=gs[:, None, :].to_broadcast([P, N_GROUP, N_GROUP]),
                        in1=gs[:, :, None].to_broadcast([P, N_GROUP, N_GROUP]),
                        op=ALU.is_gt)
                    c8 = mpool.tile([P, N_GROUP], f32, tag="c8")
                    nc.vector.tensor_reduce(out=c8, in_=cc, axis=AX.X, op=ALU.add)

                    # ---- additive mask: 0 for selected groups, -1e30 else ----
                    madd = mpool.tile([P, N_GROUP], f32, tag="madd")
                    nc.vector.scalar_tensor_tensor(
                        madd, c8, float(TOPK_GROUP) - 0.5,
                        negbig.to_broadcast([P, N_GROUP]),
                        op0=ALU.is_gt, op1=ALU.mult)

                    msf = spool.tile([P, E], f32, tag="msf")
                    nc.vector.tensor_add(
                        msf.rearrange("p (g e) -> p g e", g=N_GROUP),
                        sb_.rearrange("p (g e) -> p g e", g=N_GROUP),
                        madd[:, :, None].to_broadcast([P, N_GROUP, GSIZE]))

                    # ---- top-8 of masked scores ----
                    v8 = mpool.tile([P, 8], f32, tag="v8")
                    nc.vector.max(out=v8, in_=msf)
                    nc.vector.max_index(out=oidx_sb[:, tl, :], in_max=v8, in_values=msf)

                    # ---- selected sigmas: (msf >= v8[7]) * sigma ----
                    ssel = spool.tile([P, E], f32, tag="ssel")
                    nc.vector.scalar_tensor_tensor(
                        ssel, msf, v8[:, 7:8], sg, op0=ALU.is_ge, op1=ALU.mult)
                    s8 = mpool.tile([P, 8], f32, tag="s8")
                    nc.vector.max(out=s8, in_=ssel)
                    i8 = mpool.tile([P, 8], u32, tag="i8")
                    nc.vector.max_index(out=i8, in_max=s8, in_values=ssel)

                    # ---- reorder sigmas to score-rank order via 8x8 match ----
                    eq = mpool.tile([P, 8, 8], f32, tag="eq")
                    nc.vector.tensor_tensor(
                        out=eq,
                        in0=oidx_sb[:, tl, :, None].to_broadcast([P, 8, 8]),
                        in1=i8[:, None, :].to_broadcast([P, 8, 8]),
                        op=ALU.is_equal)
                    sr3 = mpool.tile([P, 8, 8], f32, tag="sr3")
                    nc.vector.tensor_tensor(
                        out=sr3, in0=eq,
                        in1=s8[:, None, :].to_broadcast([P, 8, 8]),
                        op=ALU.mult)
                    srank = mpool.tile([P, 8], f32, tag="srank")
                    nc.vector.tensor_reduce(out=srank, in_=sr3, axis=AX.X, op=ALU.add)

                    # ---- normalize * 2.5 ----
                    ssum = mpool.tile([P, 1], f32, tag="ssum")
                    nc.vector.tensor_reduce(out=ssum, in_=srank, axis=AX.X, op=ALU.add)
                    rs = mpool.tile([P, 1], f32, tag="rs")
                    nc.vector.reciprocal(rs, ssum)
                    nc.vector.tensor_scalar(
                        out=owgt_sb[:, tl, :], in0=srank,
                        scalar1=rs, scalar2=SCALING, op0=ALU.mult, op1=ALU.mult)

            nc.sync.dma_start(d_oidx.ap(), oidx_sb)
            nc.sync.dma_start(d_owgt.ap(), owgt_sb)

    nc.compile()
    return nc


def _get_nc(mode):
    if mode not in _CACHED_NC:
        _CACHED_NC[mode] = build_kernel(mode)
    return _CACHED_NC[mode]


def kernel(hidden_states, w, e_score_correction_bias, mode=MATMUL_MODE):
    T = hidden_states.shape[0] * hidden_states.shape[1]
    assert T == N_CORES * T_CORE
    x2 = np.ascontiguousarray(hidden_states.reshape(T, H).astype(np.float32))
    xT = np.ascontiguousarray(x2.T)                       # [H, T]
    bias_rep = np.ascontiguousarray(
        np.repeat(np.asarray(e_score_correction_bias, np.float32)[None, :], P, 0))

    if mode == "fp32":
        wT = np.ascontiguousarray(np.asarray(w, np.float32).T)  # [H, E]
        xs = {"xT": xT}
        ws = {"wT": wT}
    else:
        wT = np.ascontiguousarray(np.asarray(w, np.float32).T)
        xh = xT.astype(np.float16)
        xl = ((xT - xh.astype(np.float32)) * LO_SCALE).astype(np.float16)
        whh = wT.astype(np.float16)
        wll = ((wT - whh.astype(np.float32)) * LO_SCALE).astype(np.float16)
        xs = {"xTh": xh, "xTl": xl}
        ws = {"wTh": whh, "wTl": wll}

    nc = _get_nc(mode)
    in_maps = []
    for c in range(N_CORES):
        m = {k: np.ascontiguousarray(v[:, c * T_CORE:(c + 1) * T_CORE])
             for k, v in xs.items()}
        m.update(ws)
        m["biasrep"] = bias_rep
        in_maps.append(m)

    res = run_bass_kernel_spmd(nc, in_maps, core_ids=list(range(N_CORES)))

    idx_parts, wgt_parts = [], []
    for c in range(N_CORES):
        r = res.results[c]
        idx_parts.append(r["oidx"].transpose(1, 0, 2).reshape(T_CORE, TOP_K))
        wgt_parts.append(r["owgt"].transpose(1, 0, 2).reshape(T_CORE, TOP_K))
    topk_idx = np.concatenate(idx_parts, 0).astype(np.int32)
    topk_weight = np.concatenate(wgt_parts, 0).astype(np.float32)
    return topk_idx, topk_weight


# revision 5
# speedup vs baseline: 1.1846x; 1.1846x over previous
"""Trainium2 Bass kernel for DeepSeek-style MoE gate routing.

hidden_states [8, 4096, 2048] f32, w [256, 2048] f32, bias [256] f32
 -> topk_idx [32768, 8] int32, topk_weight [32768, 8] f32

Sharding: tokens split 8 ways across NeuronCores (4096 tokens/core); the
small gate weight + bias are replicated.  x is pre-transposed on the host so
the hidden dim lands on SBUF partitions with fully-contiguous DMA.

Matmul modes:
  fp32        - native fp32 matmuls (4 cyc/row).
  split3_bf16 - x and w split host-side into bf16 hi + bf16 lo;
                logits = xh*wh + xh*wl + xl*wh accumulated in one PSUM
                bank.  ~fp32-grade routing at bf16 matmul rate.

Self-contained: hardcodes all shapes; only imports the concourse toolchain.
"""
import sys

if "/opt/trn_rl_repo" not in sys.path:
    sys.path.insert(0, "/opt/trn_rl_repo")

import numpy as np

import concourse.bass as bass  # noqa: F401
import concourse.mybir as mybir
import concourse.tile as tile
from concourse import bacc
from concourse.bass_utils import run_bass_kernel_spmd

P = 128            # partitions / tokens per tile
H = 2048           # hidden dim
E = 256            # experts
KO = H // P        # 16 contraction chunks
N_CORES = 8
T_CORE = 4096      # tokens per core
N_TILES = T_CORE // P       # 32 token tiles per core
ST_TOK = 512                # tokens per super-tile
N_ST = T_CORE // ST_TOK     # 8 super-tiles
TPS = ST_TOK // P           # 4 tiles per super-tile

N_GROUP = 8
GSIZE = E // N_GROUP        # 32
TOPK_GROUP = 4
TOP_K = 8
SCALING = 2.5
NEG_BIG = -1.0e30

MATMUL_MODE = "split3_bf16"

f32 = mybir.dt.float32
f16 = mybir.dt.float16
bf16 = mybir.dt.bfloat16
u32 = mybir.dt.uint32
ALU = mybir.AluOpType
ACTF = mybir.ActivationFunctionType
AX = mybir.AxisListType

_CACHED_NC = {}


def build_kernel(mode=MATMUL_MODE):
    nc = bacc.Bacc("TRN2", target_bir_lowering=False, debug=False)

    if mode == "fp32":
        d_x = [nc.dram_tensor("xT", [H, T_CORE], f32, kind="ExternalInput")]
        d_w = [nc.dram_tensor("wT", [H, E], f32, kind="ExternalInput")]
        xdt = f32
    elif mode == "split3_bf16":
        d_x = [nc.dram_tensor("xTh", [H, T_CORE], bf16, kind="ExternalInput"),
               nc.dram_tensor("xTl", [H, T_CORE], bf16, kind="ExternalInput")]
        d_w = [nc.dram_tensor("wTh", [H, E], bf16, kind="ExternalInput"),
               nc.dram_tensor("wTl", [H, E], bf16, kind="ExternalInput")]
        xdt = bf16
    else:
        raise ValueError(mode)
    d_bias = nc.dram_tensor("biasrep", [P, E], f32, kind="ExternalInput")
    d_oidx = nc.dram_tensor("oidx", [P, N_TILES, TOP_K], u32, kind="ExternalOutput")
    d_owgt = nc.dram_tensor("owgt", [P, N_TILES, TOP_K], f32, kind="ExternalOutput")

    with tile.TileContext(nc) as tc:
        with tc.tile_pool(name="const", bufs=1) as cpool, \
             tc.tile_pool(name="xin", bufs=2) as xpool, \
             tc.tile_pool(name="score", bufs=2) as spool, \
             tc.tile_pool(name="small", bufs=2) as mpool, \
             tc.tile_pool(name="psum", bufs=4, space="PSUM") as ppool:

            # ---- constants ----
            if mode == "fp32":
                w_sb = cpool.tile([P, KO, E], f32, name="w0")
                nc.sync.dma_start(w_sb, d_w[0].ap().rearrange("(ko p) e -> p ko e", p=P))
            else:
                whl = cpool.tile([P, KO, 2 * E], bf16, name="whl")
                nc.sync.dma_start(whl[:, :, :E],
                                  d_w[0].ap().rearrange("(ko p) e -> p ko e", p=P))
                nc.sync.dma_start(whl[:, :, E:],
                                  d_w[1].ap().rearrange("(ko p) e -> p ko e", p=P))
            bias_sb = cpool.tile([P, E], f32)
            nc.sync.dma_start(bias_sb, d_bias.ap())
            negbig = cpool.tile([P, 1], f32)
            nc.vector.memset(negbig, NEG_BIG)
            mask_hi = cpool.tile([P, 1], u32)
            nc.vector.memset(mask_hi, 0xFFFFFF00)
            mask_lo = cpool.tile([P, 1], u32)
            nc.vector.memset(mask_lo, 0xFF)
            iota_e = cpool.tile([P, E], u32)
            nc.gpsimd.iota(iota_e, pattern=[[1, E]], base=0, channel_multiplier=0)
            oidx_sb = cpool.tile([P, N_TILES, TOP_K], u32)
            owgt_sb = cpool.tile([P, N_TILES, TOP_K], f32)

            for st in range(N_ST):
                x_sb = []
                for i, d in enumerate(d_x):
                    t = xpool.tile([P, KO, ST_TOK], xdt, tag=f"x{i}")
                    nc.sync.dma_start(
                        t, d.ap().rearrange("(ko p) t -> p ko t", p=P)[
                            :, :, st * ST_TOK:(st + 1) * ST_TOK])
                    x_sb.append(t)

                # super-tile score tensors [128, 4, 256]
                sg_st = spool.tile([P, TPS, E], f32, tag="sg")
                sb_st = spool.tile([P, TPS, E], f32, tag="sb")
                sq_st = spool.tile([P, TPS, E], f32, tag="sq")
                msf_st = spool.tile([P, TPS, E], f32, tag="msf")
                zap_st = spool.tile([P, TPS, E], f32, tag="zap")
                ssel_st = spool.tile([P, TPS, E], f32, tag="ssel")
                t1g = mpool.tile([P, TPS, N_GROUP], f32, tag="t1g")
                t2g = mpool.tile([P, TPS, N_GROUP], f32, tag="t2g")
                gs = mpool.tile([P, TPS, N_GROUP], f32, tag="gs")
                cc = mpool.tile([P, TPS, N_GROUP, N_GROUP], f32, tag="cc")
                c8 = mpool.tile([P, TPS, N_GROUP], f32, tag="c8")
                madd = mpool.tile([P, TPS, N_GROUP], f32, tag="madd")
                v8 = mpool.tile([P, TPS, 8], f32, tag="v8")
                s8 = mpool.tile([P, TPS, 8], f32, tag="s8")
                is8 = mpool.tile([P, TPS, 8], u32, tag="is8")
                eq = mpool.tile([P, TPS, 8, 8], f32, tag="eq")
                sr3 = mpool.tile([P, TPS, 8, 8], f32, tag="sr3")
                srank = mpool.tile([P, TPS, 8], f32, tag="srank")
                ssum = mpool.tile([P, TPS, 1], f32, tag="ssum")
                rs = mpool.tile([P, TPS, 1], f32, tag="rs")

                for j in range(TPS):
                    tl = st * TPS + j
                    tsl = slice(j * P, (j + 1) * P)

                    # ---- logits ----
                    if mode == "fp32":
                        ps = ppool.tile([P, E], f32, tag="ps")
                        for k in range(KO):
                            nc.tensor.matmul(
                                ps, lhsT=x_sb[0][:, k, tsl], rhs=w_sb[:, k, :],
                                start=(k == 0), stop=(k == KO - 1))
                        sig_src = ps
                    else:
                        ps = ppool.tile([P, E], f32, tag="ps")
                        xh, xl = x_sb
                        for k in range(KO):
                            # all three split products accumulate into one bank
                            nc.tensor.matmul(
                                ps, lhsT=xh[:, k, tsl], rhs=whl[:, k, :E],
                                start=(k == 0), stop=False)
                            nc.tensor.matmul(
                                ps, lhsT=xh[:, k, tsl], rhs=whl[:, k, E:],
                                start=False, stop=False)
                            nc.tensor.matmul(
                                ps, lhsT=xl[:, k, tsl], rhs=whl[:, k, :E],
                                start=False, stop=(k == KO - 1))
                        sig_src = ps

                    # ---- sigma = sigmoid(logits) on ACT ----
                    nc.scalar.activation(sg_st[:, j, :], sig_src, ACTF.Sigmoid)

                    # scores_for_choice = sigma + bias            (GPSIMD)
                    nc.gpsimd.tensor_add(sb_st[:, j, :], sg_st[:, j, :], bias_sb)

                # sigma_q: low 8 mantissa bits <- expert id (batched DVE)
                nc.vector.scalar_tensor_tensor(
                    sq_st.bitcast(u32), sg_st.bitcast(u32),
                    mask_hi, iota_e[:, None, :].to_broadcast([P, TPS, E]),
                    op0=ALU.bitwise_and, op1=ALU.bitwise_or)

                # ---- group top-2 (batched reduce + per-tile match_replace) ----
                sb4 = sb_st.rearrange("p t (g e) -> p t g e", g=N_GROUP)
                nc.vector.tensor_reduce(out=t1g, in_=sb4, axis=AX.X, op=ALU.max)
                for j in range(TPS):
                    nc.vector.match_replace(
                        out=zap_st[:, j, :], in_to_replace=t1g[:, j, :],
                        in_values=sb_st[:, j, :], imm_value=NEG_BIG)
                nc.vector.tensor_reduce(
                    out=t2g, in_=zap_st.rearrange("p t (g e) -> p t g e", g=N_GROUP),
                    axis=AX.X, op=ALU.max)
                nc.vector.tensor_add(gs, t1g, t2g)

                # ---- group rank count + additive mask ----
                nc.vector.tensor_tensor(
                    out=cc,
                    in0=gs[:, :, None, :].to_broadcast([P, TPS, N_GROUP, N_GROUP]),
                    in1=gs[:, :, :, None].to_broadcast([P, TPS, N_GROUP, N_GROUP]),
                    op=ALU.is_gt)
                nc.vector.tensor_reduce(out=c8, in_=cc, axis=AX.X, op=ALU.add)
                nc.vector.scalar_tensor_tensor(
                    madd, c8, float(TOPK_GROUP) - 0.5,
                    negbig[:, :, None].to_broadcast([P, TPS, N_GROUP]),
                    op0=ALU.is_gt, op1=ALU.mult)

                # ---- masked scores ----
                nc.vector.tensor_add(
                    msf_st.rearrange("p t (g e) -> p t g e", g=N_GROUP),
                    sb4,
                    madd[:, :, :, None].to_broadcast([P, TPS, N_GROUP, GSIZE]))

                for j in range(TPS):
                    tl = st * TPS + j
                    # ---- top-8 of masked scores ----
                    nc.vector.max(out=v8[:, j, :], in_=msf_st[:, j, :])
                    nc.vector.max_index(out=oidx_sb[:, tl, :], in_max=v8[:, j, :],
                                        in_values=msf_st[:, j, :])
                    # ---- selected sigma_q: (msf >= v8[7]) * sigma_q ----
                    nc.vector.scalar_tensor_tensor(
                        ssel_st[:, j, :], msf_st[:, j, :], v8[:, j, 7:8],
                        sq_st[:, j, :], op0=ALU.is_ge, op1=ALU.mult)
                    nc.vector.max(out=s8[:, j, :], in_=ssel_st[:, j, :])

                # ---- decode embedded ids, reorder sigmas to score-rank order ----
                nc.vector.tensor_scalar(
                    out=is8, in0=s8.bitcast(u32), scalar1=mask_lo, scalar2=None,
                    op0=ALU.bitwise_and)
                nc.vector.tensor_tensor(
                    out=eq,
                    in0=oidx_sb[:, st * TPS:(st + 1) * TPS, :, None]
                        .to_broadcast([P, TPS, 8, 8]),
                    in1=is8[:, :, None, :].to_broadcast([P, TPS, 8, 8]),
                    op=ALU.is_equal)
                nc.vector.tensor_tensor(
                    out=sr3, in0=eq,
                    in1=s8[:, :, None, :].to_broadcast([P, TPS, 8, 8]),
                    op=ALU.mult)
                nc.vector.tensor_reduce(out=srank, in_=sr3, axis=AX.X, op=ALU.add)

                # ---- normalize * 2.5 ----
                nc.vector.tensor_reduce(out=ssum, in_=srank, axis=AX.X, op=ALU.add)
                nc.vector.reciprocal(rs, ssum)
                nc.vector.scalar_tensor_tensor(
                    owgt_sb[:, st * TPS:(st + 1) * TPS, :], srank, SCALING,
                    rs.to_broadcast([P, TPS, 8]),
                    op0=ALU.mult, op1=ALU.mult)

            nc.sync.dma_start(d_oidx.ap(), oidx_sb)
            nc.sync.dma_start(d_owgt.ap(), owgt_sb)

    nc.compile()
    return nc


def _get_nc(mode):
    if mode not in _CACHED_NC:
        _CACHED_NC[mode] = build_kernel(mode)
    return _CACHED_NC[mode]


def kernel(hidden_states, w, e_score_correction_bias, mode=MATMUL_MODE):
    T = hidden_states.shape[0] * hidden_states.shape[1]
    assert T == N_CORES * T_CORE
    x2 = np.ascontiguousarray(hidden_states.reshape(T, H).astype(np.float32))
    xT = np.ascontiguousarray(x2.T)                       # [H, T]
    wT = np.ascontiguousarray(np.asarray(w, np.float32).T)  # [H, E]
    bias_rep = np.ascontiguousarray(
        np.repeat(np.asarray(e_score_correction_bias, np.float32)[None, :], P, 0))

    if mode == "fp32":
        xs = {"xT": xT}
        ws = {"wT": wT}
    else:
        import ml_dtypes
        bf = ml_dtypes.bfloat16
        xh = xT.astype(bf)
        xl = (xT - xh.astype(np.float32)).astype(bf)
        whh = wT.astype(bf)
        wll = (wT - whh.astype(np.float32)).astype(bf)
        xs = {"xTh": xh, "xTl": xl}
        ws = {"wTh": whh, "wTl": wll}

    nc = _get_nc(mode)
    in_maps = []
    for c in range(N_CORES):
        m = {k: np.ascontiguousarray(v[:, c * T_CORE:(c + 1) * T_CORE])
             for k, v in xs.items()}
        m.update(ws)
        m["biasrep"] = bias_rep
        in_maps.append(m)

    res = run_bass_kernel_spmd(nc, in_maps, core_ids=list(range(N_CORES)))

    idx_parts, wgt_parts = [], []
    for c in range(N_CORES):
        r = res.results[c]
        idx_parts.append(r["oidx"].transpose(1, 0, 2).reshape(T_CORE, TOP_K))
        wgt_parts.append(r["owgt"].transpose(1, 0, 2).reshape(T_CORE, TOP_K))
    topk_idx = np.concatenate(idx_parts, 0).astype(np.int32)
    topk_weight = np.concatenate(wgt_parts, 0).astype(np.float32)
    return topk_idx, topk_weight


# revision 6
# speedup vs baseline: 1.1927x; 1.0069x over previous
"""Trainium2 Bass kernel for DeepSeek-style MoE gate routing.

hidden_states [8, 4096, 2048] f32, w [256, 2048] f32, bias [256] f32
 -> topk_idx [32768, 8] int32, topk_weight [32768, 8] f32

Sharding: tokens split 8 ways across NeuronCores (4096 tokens/core); the
small gate weight + bias are replicated.  x is pre-transposed on the host so
the hidden dim lands on SBUF partitions with fully-contiguous DMA.

Matmul modes:
  fp32        - native fp32 matmuls (4 cyc/row).
  split3_bf16 - x and w split host-side into bf16 hi + bf16 lo;
                logits = xh*wh + xh*wl + xl*wh accumulated in one PSUM
                bank.  ~fp32-grade routing at bf16 matmul rate.

Self-contained: hardcodes all shapes; only imports the concourse toolchain.
"""
import sys

if "/opt/trn_rl_repo" not in sys.path:
    sys.path.insert(0, "/opt/trn_rl_repo")

import numpy as np

import concourse.bass as bass  # noqa: F401
import concourse.mybir as mybir
import concourse.tile as tile
from concourse import bacc
from concourse.bass_utils import run_bass_kernel_spmd

P = 128            # partitions / tokens per tile
H = 2048           # hidden dim
E = 256            # experts
KO = H // P        # 16 contraction chunks
N_CORES = 8
T_CORE = 4096      # tokens per core
N_TILES = T_CORE // P       # 32 token tiles per core
ST_TOK = 512                # tokens per super-tile
N_ST = T_CORE // ST_TOK     # 8 super-tiles
TPS = ST_TOK // P           # 4 tiles per super-tile

N_GROUP = 8
GSIZE = E // N_GROUP        # 32
TOPK_GROUP = 4
TOP_K = 8
SCALING = 2.5
NEG_BIG = -1.0e30

MATMUL_MODE = "split3_bf16"

f32 = mybir.dt.float32
f16 = mybir.dt.float16
bf16 = mybir.dt.bfloat16
u32 = mybir.dt.uint32
ALU = mybir.AluOpType
ACTF = mybir.ActivationFunctionType
AX = mybir.AxisListType

_CACHED_NC = {}


def build_kernel(mode=MATMUL_MODE):
    nc = bacc.Bacc("TRN2", target_bir_lowering=False, debug=False)

    if mode == "fp32":
        d_x = [nc.dram_tensor("xT", [H, T_CORE], f32, kind="ExternalInput")]
        d_w = [nc.dram_tensor("wT", [H, E], f32, kind="ExternalInput")]
        xdt = f32
    elif mode == "split3_bf16":
        d_x = [nc.dram_tensor("xTh", [H, T_CORE], bf16, kind="ExternalInput"),
               nc.dram_tensor("xTl", [H, T_CORE], bf16, kind="ExternalInput")]
        d_w = [nc.dram_tensor("wTh", [H, E], bf16, kind="ExternalInput"),
               nc.dram_tensor("wTl", [H, E], bf16, kind="ExternalInput")]
        xdt = bf16
    else:
        raise ValueError(mode)
    d_bias = nc.dram_tensor("biasrep", [P, E], f32, kind="ExternalInput")
    d_oidx = nc.dram_tensor("oidx", [P, N_TILES, TOP_K], u32, kind="ExternalOutput")
    d_owgt = nc.dram_tensor("owgt", [P, N_TILES, TOP_K], f32, kind="ExternalOutput")

    with tile.TileContext(nc) as tc:
        with tc.tile_pool(name="const", bufs=1) as cpool, \
             tc.tile_pool(name="xin", bufs=2) as xpool, \
             tc.tile_pool(name="score", bufs=2) as spool, \
             tc.tile_pool(name="small", bufs=2) as mpool, \
             tc.tile_pool(name="psum", bufs=4, space="PSUM") as ppool:

            # ---- constants ----
            if mode == "fp32":
                w_sb = cpool.tile([P, KO, E], f32, name="w0")
                nc.sync.dma_start(w_sb, d_w[0].ap().rearrange("(ko p) e -> p ko e", p=P))
            else:
                whl = cpool.tile([P, KO, 2 * E], bf16, name="whl")
                nc.sync.dma_start(whl[:, :, :E],
                                  d_w[0].ap().rearrange("(ko p) e -> p ko e", p=P))
                nc.sync.dma_start(whl[:, :, E:],
                                  d_w[1].ap().rearrange("(ko p) e -> p ko e", p=P))
            bias_sb = cpool.tile([P, E], f32)
            nc.sync.dma_start(bias_sb, d_bias.ap())
            negbig = cpool.tile([P, 1], f32)
            nc.vector.memset(negbig, NEG_BIG)
            mask_hi = cpool.tile([P, 1], u32)
            nc.vector.memset(mask_hi, 0xFFFFFF00)
            mask_lo = cpool.tile([P, 1], u32)
            nc.vector.memset(mask_lo, 0xFF)
            iota_e = cpool.tile([P, E], u32)
            nc.gpsimd.iota(iota_e, pattern=[[1, E]], base=0, channel_multiplier=0)
            oidx_sb = cpool.tile([P, N_TILES, TOP_K], u32)
            owgt_sb = cpool.tile([P, N_TILES, TOP_K], f32)

            for st in range(N_ST):
                x_sb = []
                for i, d in enumerate(d_x):
                    t = xpool.tile([P, KO, ST_TOK], xdt, tag=f"x{i}")
                    half = ST_TOK // 2
                    src_ap = d.ap().rearrange("(ko p) t -> p ko t", p=P)
                    for hh in range(2):
                        nc.sync.dma_start(
                            t[:, :, hh * half:(hh + 1) * half],
                            src_ap[:, :, st * ST_TOK + hh * half:
                                   st * ST_TOK + (hh + 1) * half])
                    x_sb.append(t)

                # super-tile score tensors [128, 4, 256]
                sg_st = spool.tile([P, TPS, E], f32, tag="sg")
                sb_st = spool.tile([P, TPS, E], f32, tag="sb")
                sq_st = spool.tile([P, TPS, E], f32, tag="sq")
                msf_st = spool.tile([P, TPS, E], f32, tag="msf")
                zap_st = spool.tile([P, TPS, E], f32, tag="zap")
                ssel_st = spool.tile([P, TPS, E], f32, tag="ssel")
                t1g = mpool.tile([P, TPS, N_GROUP], f32, tag="t1g")
                t2g = mpool.tile([P, TPS, N_GROUP], f32, tag="t2g")
                gs = mpool.tile([P, TPS, N_GROUP], f32, tag="gs")
                cc = mpool.tile([P, TPS, N_GROUP, N_GROUP], f32, tag="cc")
                c8 = mpool.tile([P, TPS, N_GROUP], f32, tag="c8")
                madd = mpool.tile([P, TPS, N_GROUP], f32, tag="madd")
                v8 = mpool.tile([P, TPS, 8], f32, tag="v8")
                s8 = mpool.tile([P, TPS, 8], f32, tag="s8")
                is8 = mpool.tile([P, TPS, 8], u32, tag="is8")
                eq = mpool.tile([P, TPS, 8, 8], f32, tag="eq")
                sr3 = mpool.tile([P, TPS, 8, 8], f32, tag="sr3")
                srank = mpool.tile([P, TPS, 8], f32, tag="srank")
                ssum = mpool.tile([P, TPS, 1], f32, tag="ssum")
                rs = mpool.tile([P, TPS, 1], f32, tag="rs")

                for j in range(TPS):
                    tl = st * TPS + j
                    tsl = slice(j * P, (j + 1) * P)

                    # ---- logits ----
                    if mode == "fp32":
                        ps = ppool.tile([P, E], f32, tag="ps")
                        for k in range(KO):
                            nc.tensor.matmul(
                                ps, lhsT=x_sb[0][:, k, tsl], rhs=w_sb[:, k, :],
                                start=(k == 0), stop=(k == KO - 1))
                        sig_src = ps
                    else:
                        ps = ppool.tile([P, E], f32, tag="ps")
                        xh, xl = x_sb
                        for k in range(KO):
                            # all three split products accumulate into one bank
                            nc.tensor.matmul(
                                ps, lhsT=xh[:, k, tsl], rhs=whl[:, k, :E],
                                start=(k == 0), stop=False)
                            nc.tensor.matmul(
                                ps, lhsT=xh[:, k, tsl], rhs=whl[:, k, E:],
                                start=False, stop=False)
                            nc.tensor.matmul(
                                ps, lhsT=xl[:, k, tsl], rhs=whl[:, k, :E],
                                start=False, stop=(k == KO - 1))
                        sig_src = ps

                    # ---- sigma = sigmoid(logits) on ACT ----
                    nc.scalar.activation(sg_st[:, j, :], sig_src, ACTF.Sigmoid)

                    # scores_for_choice = sigma + bias            (GPSIMD)
                    nc.gpsimd.tensor_add(sb_st[:, j, :], sg_st[:, j, :], bias_sb)

                # sigma_q: low 8 mantissa bits <- expert id (batched DVE)
                nc.vector.scalar_tensor_tensor(
                    sq_st.bitcast(u32), sg_st.bitcast(u32),
                    mask_hi, iota_e[:, None, :].to_broadcast([P, TPS, E]),
                    op0=ALU.bitwise_and, op1=ALU.bitwise_or)

                # ---- group top-2 (batched reduce + per-tile match_replace) ----
                sb4 = sb_st.rearrange("p t (g e) -> p t g e", g=N_GROUP)
                nc.vector.tensor_reduce(out=t1g, in_=sb4, axis=AX.X, op=ALU.max)
                for j in range(TPS):
                    nc.vector.match_replace(
                        out=zap_st[:, j, :], in_to_replace=t1g[:, j, :],
                        in_values=sb_st[:, j, :], imm_value=NEG_BIG)
                nc.vector.tensor_reduce(
                    out=t2g, in_=zap_st.rearrange("p t (g e) -> p t g e", g=N_GROUP),
                    axis=AX.X, op=ALU.max)
                nc.vector.tensor_add(gs, t1g, t2g)

                # ---- group rank count + additive mask ----
                nc.vector.tensor_tensor(
                    out=cc,
                    in0=gs[:, :, None, :].to_broadcast([P, TPS, N_GROUP, N_GROUP]),
                    in1=gs[:, :, :, None].to_broadcast([P, TPS, N_GROUP, N_GROUP]),
                    op=ALU.is_gt)
                nc.vector.tensor_reduce(out=c8, in_=cc, axis=AX.X, op=ALU.add)
                nc.vector.scalar_tensor_tensor(
                    madd, c8, float(TOPK_GROUP) - 0.5,
                    negbig[:, :, None].to_broadcast([P, TPS, N_GROUP]),
                    op0=ALU.is_gt, op1=ALU.mult)

                # ---- masked scores ----
                nc.vector.tensor_add(
                    msf_st.rearrange("p t (g e) -> p t g e", g=N_GROUP),
                    sb4,
                    madd[:, :, :, None].to_broadcast([P, TPS, N_GROUP, GSIZE]))

                for j in range(TPS):
                    tl = st * TPS + j
                    # ---- top-8 of masked scores ----
                    nc.vector.max(out=v8[:, j, :], in_=msf_st[:, j, :])
                    nc.vector.max_index(out=oidx_sb[:, tl, :], in_max=v8[:, j, :],
                                        in_values=msf_st[:, j, :])
                    # ---- selected sigma_q: (msf >= v8[7]) * sigma_q ----
                    nc.vector.scalar_tensor_tensor(
                        ssel_st[:, j, :], msf_st[:, j, :], v8[:, j, 7:8],
                        sq_st[:, j, :], op0=ALU.is_ge, op1=ALU.mult)
                    nc.vector.max(out=s8[:, j, :], in_=ssel_st[:, j, :])

                # ---- decode embedded ids, reorder sigmas to score-rank order ----
                nc.vector.tensor_scalar(
                    out=is8, in0=s8.bitcast(u32), scalar1=mask_lo, scalar2=None,
                    op0=ALU.bitwise_and)
                nc.vector.tensor_tensor(
                    out=eq,
                    in0=oidx_sb[:, st * TPS:(st + 1) * TPS, :, None]
                        .to_broadcast([P, TPS, 8, 8]),
                    in1=is8[:, :, None, :].to_broadcast([P, TPS, 8, 8]),
                    op=ALU.is_equal)
                nc.vector.tensor_tensor(
                    out=sr3, in0=eq,
                    in1=s8[:, :, None, :].to_broadcast([P, TPS, 8, 8]),
                    op=ALU.mult)
                nc.vector.tensor_reduce(out=srank, in_=sr3, axis=AX.X, op=ALU.add)

                # ---- normalize * 2.5 ----
                nc.vector.tensor_reduce(out=ssum, in_=srank, axis=AX.X, op=ALU.add)
                nc.vector.reciprocal(rs, ssum)
                nc.vector.scalar_tensor_tensor(
                    owgt_sb[:, st * TPS:(st + 1) * TPS, :], srank, SCALING,
                    rs.to_broadcast([P, TPS, 8]),
                    op0=ALU.mult, op1=ALU.mult)

                ssl = slice(st * TPS, (st + 1) * TPS)
                nc.sync.dma_start(d_oidx.ap()[:, ssl, :], oidx_sb[:, ssl, :])
                nc.sync.dma_start(d_owgt.ap()[:, ssl, :], owgt_sb[:, ssl, :])

    nc.compile()
    return nc


def _get_nc(mode):
    if mode not in _CACHED_NC:
        _CACHED_NC[mode] = build_kernel(mode)
    return _CACHED_NC[mode]


def kernel(hidden_states, w, e_score_correction_bias, mode=MATMUL_MODE):
    T = hidden_states.shape[0] * hidden_states.shape[1]
    assert T == N_CORES * T_CORE
    x2 = np.ascontiguousarray(hidden_states.reshape(T, H).astype(np.float32))
    xT = np.ascontiguousarray(x2.T)                       # [H, T]
    wT = np.ascontiguousarray(np.asarray(w, np.float32).T)  # [H, E]
    bias_rep = np.ascontiguousarray(
        np.repeat(np.asarray(e_score_correction_bias, np.float32)[None, :], P, 0))

    if mode == "fp32":
        xs = {"xT": xT}
        ws = {"wT": wT}
    else:
        import ml_dtypes
        bf = ml_dtypes.bfloat16
        xh = xT.astype(bf)
        xl = (xT - xh.astype(np.float32)).astype(bf)
        whh = wT.astype(bf)
        wll = (wT - whh.astype(np.float32)).astype(bf)
        xs = {"xTh": xh, "xTl": xl}
        ws = {"wTh": whh, "wTl": wll}

    nc = _get_nc(mode)
    in_maps = []
    for c in range(N_CORES):
        m = {k: np.ascontiguousarray(v[:, c * T_CORE:(c + 1) * T_CORE])
             for k, v in xs.items()}
        m.update(ws)
        m["biasrep"] = bias_rep
        in_maps.append(m)

    res = run_bass_kernel_spmd(nc, in_maps, core_ids=list(range(N_CORES)))

    idx_parts, wgt_parts = [], []
    for c in range(N_CORES):
        r = res.results[c]
        idx_parts.append(r["oidx"].transpose(1, 0, 2).reshape(T_CORE, TOP_K))
        wgt_parts.append(r["owgt"].transpose(1, 0, 2).reshape(T_CORE, TOP_K))
    topk_idx = np.concatenate(idx_parts, 0).astype(np.int32)
    topk_weight = np.concatenate(wgt_parts, 0).astype(np.float32)
    return topk_idx, topk_weight
